# revision 10
# baseline (speedup 1.0000x reference)
"""Trainium2 Bass kernel for nn_EqStftPBC (STFT perturbation-based compensation).

Per (batch b, mode m):
  X = STFT(x); C_n2 = X*conj(roll(X,n2)); U_n2 = circ(w[:,n2]) @ C (+ time-roll);
  V_n2 = U_n2 * roll(X,n2); delta_f = sum_n2 V_n2; D = IDFT(delta); host OLA.
8 cores = (b x m x n2-half); per-core data-only variation (S/M stacks).

v7 (from v5 ~31.5us):
- device outputs D [80, 2T] fp32; overlap-add/cov/P-scale moved to host
  (kills Y-stage mms + selector consts + D guard memsets; shorter tail).
- XtB broadcast ACT removed: C-stage reads X via 0-stride-over-j APs.
- input DMA posts moved off scalar (ACT_TABLE_LOAD no longer gates them):
  crit posted by the tensor engine itself at t~6.1us, smat by vector,
  mst/gr_c by gpsimd; sync carries ONLY the output DMA.
- component-major R/U/V layouts ([comp(520) | comp(520)]) enabling
  wide-packed C/V stages: 2 double-width MUL TTs + 2 combine TTs each
  (was 6 TTs) -- fewer DVE ops, same math.
- time-roll as before: ghost slots + one flat TT per chunk.
- G-stage j-sum in PSUM via zero-stride dst (tensor has slack vs DVE).
"""

import numpy as np
from ml_dtypes import bfloat16, float8_e4m3

import concourse.bass as bass
import concourse.bacc as bacc
import concourse.mybir as mybir
import concourse.tile as tile

F = 80
T = 51
TP = 52          # per-j slot stride (51 data + 1 pad)
HOP = 40
L = 2080
NJ = 20
NCH = 2
CHJ = NJ // NCH  # 10
PBK = 5          # j's per R/U psum bank
BL = CHJ * TP    # 520
UEC = CHJ * TP + 1   # 521: per-component Ue extent (slots + 1 tail junk)
FP32 = mybir.dt.float32
BF16 = mybir.dt.bfloat16
FP8 = mybir.dt.float8e4

N2_LISTS = [list(range(19, -1, -1)), list(range(-1, -21, -1))]


def _dft_consts():
    j = np.arange(F)
    W = np.exp(-2j * np.pi * np.outer(j, j) / F)
    G = np.exp(+2j * np.pi * np.outer(j, j) / F) / F
    return W, G


def build_program(debug=False):
    nc = bacc.Bacc("TRN2", target_bir_lowering=False, debug=debug)

    # crit = [xf frames (3T) | fr_c (2F)]: one DMA gates the STFT
    crit = nc.dram_tensor("crit", [F, 3 * T + 2 * F], BF16, kind="ExternalInput")
    # gr_c = [Gr | Gi | GiN]  (GiN = -Gi)
    gr_c = nc.dram_tensor("gr_c", [F, 3 * F], BF16, kind="ExternalInput")
    smat = nc.dram_tensor("smat", [F, NJ * F], FP8, kind="ExternalInput")
    mst = nc.dram_tensor("mst", [F, NJ * 2 * F], BF16, kind="ExternalInput")
    dv = nc.dram_tensor("dv", [F, 2 * T], FP32, kind="ExternalOutput")

    MUL = mybir.AluOpType.mult
    ADD = mybir.AluOpType.add
    SUB = mybir.AluOpType.subtract
    CPY = mybir.ActivationFunctionType.Copy

    with tile.TileContext(nc) as tc:
        with (
            tc.tile_pool(name="const", bufs=1) as cpool,
            tc.tile_pool(name="work", bufs=1) as wpool,
            tc.tile_pool(name="ps_x", bufs=1, space="PSUM") as ps_x,
            tc.tile_pool(name="ps_r", bufs=2, space="PSUM") as ps_r,
            tc.tile_pool(name="ps_u", bufs=2, space="PSUM") as ps_u,
            tc.tile_pool(name="ps_d", bufs=1, space="PSUM") as ps_d,
        ):
            # ---- input DMAs: only gpsimd/sync/scalar may post. crit alone on
            # the sync queue (fast first-post); everything else on the
            # high-bandwidth gpsimd queue, smat first (it gates the R stage);
            # scalar posts nothing so its ACTs are never queue-blocked.
            Crit = wpool.tile([F, 3 * T + 2 * F], BF16, tag="Crit")
            nc.sync.dma_start(Crit[:, :], crit[:, :])
            FCO = 3 * T   # Fc column offset within Crit
            # smat follows crit on the sync queue: sharing the gpsimd queue
            # with mst lets descriptor striping delay smat's tail by ~1.6us
            # (one straggler descriptor gates the whole R stage).
            Ssb = cpool.tile([F, NJ * F], FP8, tag="Ssb")
            nc.sync.dma_start(Ssb[:, 0:CHJ * F], smat[:, 0:CHJ * F])
            nc.sync.dma_start(Ssb[:, CHJ * F:], smat[:, CHJ * F:])
            Msb = cpool.tile([F, NJ * 2 * F], BF16, tag="Msb")
            nc.gpsimd.dma_start(Msb[:, 0:CHJ * 2 * F], mst[:, 0:CHJ * 2 * F])
            nc.gpsimd.dma_start(Msb[:, CHJ * 2 * F:], mst[:, CHJ * 2 * F:])
            Gc = cpool.tile([F, 3 * F], BF16, tag="Gc")
            nc.gpsimd.dma_start(Gc[:, :], gr_c[:, :])

            # zero rhs for the PSUM-accumulation opener matmul
            Zsb = wpool.tile([F, 2 * T], BF16, tag="Zsb")
            nc.gpsimd.memset(Zsb[:, :], 0.0)

            # ---- STFT (fp32 accum) -> X bf16 [Xr(52) | Xi(52)] ----
            Xp = ps_x.tile([F, 2 * T], FP32, tag="Xp")
            nc.tensor.matmul(Xp[:, :], Crit[:, FCO:FCO + F], Crit[:, T:3 * T],
                             start=True, stop=False)
            nc.tensor.matmul(Xp[:, :], Crit[:, FCO + F:FCO + 2 * F], Crit[:, 0:2 * T],
                             start=False, stop=True)
            Xsb = wpool.tile([F, 2 * TP], BF16, tag="Xsb")
            # pad columns of the X slots (read by the C-stage broadcast APs)
            nc.gpsimd.memset(bass.AP(tensor=Xsb[:, :].tensor,
                                     offset=Xsb[:, :].offset + T,
                                     ap=[[2 * TP, F], [TP, 2], [1, 1]]), 0.0)
            Xsv = Xsb[:, :].rearrange("p (c t) -> p c t", c=2)
            nc.scalar.activation(Xsv[:, :, 0:T],
                                 Xp[:, :].rearrange("p (c t) -> p c t", c=2), CPY)
            Xrhs = bass.AP(tensor=Xsb[:, :].tensor, offset=Xsb[:, :].offset,
                           ap=[[2 * TP, F], [TP, 2], [1, T]])

            # X broadcast APs for the C stage: (c2, j0-stride, t) and the
            # c2-reversed variant (Xi then Xr) for the P3/P4 products.
            def x_bcast(rev):
                if not rev:
                    return bass.AP(tensor=Xsb[:, :].tensor, offset=Xsb[:, :].offset,
                                   ap=[[2 * TP, F], [TP, 2], [0, CHJ], [1, TP]])
                return bass.AP(tensor=Xsb[:, :].tensor,
                               offset=Xsb[:, :].offset + TP,
                               ap=[[2 * TP, F], [-TP, 2], [0, CHJ], [1, TP]])

            # ---- per-chunk tiles (component-major: [r(520) | i(520)]) ----
            Rsb, Csb, Usb, Vsb, Ue = [], [], [], [], []
            for c in range(NCH):
                Rsb.append(wpool.tile([F, 2 * BL], BF16, tag=f"Rsb{c}", name=f"Rsb{c}"))
                Csb.append(wpool.tile([F, 3 * BL], BF16, tag=f"Csb{c}", name=f"Csb{c}"))
                Usb.append(wpool.tile([F, 2 * BL], BF16, tag=f"Usb{c}", name=f"Usb{c}"))
                Vsb.append(wpool.tile([F, 2 * BL], BF16, tag=f"Vsb{c}", name=f"Vsb{c}"))
                Ue.append(wpool.tile([F, 2 * UEC], BF16, tag=f"Ue{c}", name=f"Ue{c}"))
                # tail junk element per component (read by the roll TT pad col)
                nc.gpsimd.memset(bass.AP(tensor=Ue[c][:, :].tensor,
                                         offset=Ue[c][:, :].offset + UEC - 1,
                                         ap=[[2 * UEC, F], [UEC, 2], [1, 1]]), 0.0)
            sP = [wpool.tile([F, 2 * BL], BF16, tag=f"sP{c}", name=f"sP{c}")
                  for c in range(NCH)]
            sQ = [wpool.tile([F, 2 * BL], BF16, tag=f"sQ{c}", name=f"sQ{c}")
                  for c in range(NCH)]

            TT = nc.vector.tensor_tensor

            def r_stage(c, split_evict=False):
                """R_j = roll(X, n2_j): permutation matmuls, evict to SBUF.

                split_evict: bank B evicted by a vector tensor_copy so both
                bank evictions run concurrently (shorter pipeline fill).
                """
                Rc = Rsb[c]
                for bk in range(CHJ // PBK):
                    Rp = ps_r.tile([F, PBK * 2 * T], FP32, tag="Rp")
                    for s in range(PBK):
                        j = c * CHJ + bk * PBK + s
                        nc.tensor.matmul(Rp[:, s * 2 * T:(s + 1) * 2 * T],
                                         Ssb[:, j * F:(j + 1) * F],
                                         Xrhs, start=True, stop=True)
                    dst = bass.AP(tensor=Rc[:, :].tensor,
                                  offset=Rc[:, :].offset + bk * PBK * TP,
                                  ap=[[2 * BL, F], [TP, PBK], [BL, 2], [1, T]])
                    src = Rp[:, :].rearrange("p (s c2 t) -> p s c2 t", s=PBK, c2=2)
                    if split_evict and bk == 1:
                        nc.vector.tensor_copy(dst, src)
                    else:
                        nc.scalar.activation(dst, src, CPY)

            def rf(tile_, rev=False, off=0):
                """[comp0 | comp1] (or reversed) view of a component-major tile."""
                if not rev:
                    return bass.AP(tensor=tile_[:, :].tensor,
                                   offset=tile_[:, :].offset + off,
                                   ap=[[2 * BL, F], [BL, 2], [1, BL]])
                return bass.AP(tensor=tile_[:, :].tensor,
                               offset=tile_[:, :].offset + BL + off,
                               ap=[[2 * BL, F], [-BL, 2], [1, BL]])

            def c_stage(c):
                """C = X * conj(R) -> Csb [CiN | Cr | Ci]; wide-packed TTs.

                sP = [P1|P2] = [Xr*Rr | Xi*Ri]; sQ = [P3|P4] = [Xi*Rr | Xr*Ri]
                Cr = P1+P2 ; CiN = P4-P3 ; Ci = -CiN (scalar).
                """
                Rc, Cc = Rsb[c], Csb[c]
                TT(sQ[c][:, :], x_bcast(True), rf(Rc), MUL)
                TT(Cc[:, 0:BL], sQ[c][:, BL:2 * BL], sQ[c][:, 0:BL], SUB)
                nc.scalar.activation(Cc[:, 2 * BL:3 * BL], Cc[:, 0:BL], CPY,
                                     scale=-1.0)
                TT(sP[c][:, :], x_bcast(False), rf(Rc), MUL)
                TT(Cc[:, BL:2 * BL], sP[c][:, 0:BL], sP[c][:, BL:2 * BL], ADD)

            def u_mm(c):
                """Up_j = Mi@[CiN|Cr] + Mr@[Cr|Ci]; evict into ghost-slotted Ue."""
                Cc = Csb[c]
                for bk in range(CHJ // PBK):
                    Up = ps_u.tile([F, PBK * 2 * T], FP32, tag="Up")
                    for s in range(PBK):
                        jj = bk * PBK + s
                        j = c * CHJ + jj
                        rhs1 = bass.AP(tensor=Cc[:, :].tensor,
                                       offset=Cc[:, :].offset + BL + jj * TP,
                                       ap=[[3 * BL, F], [BL, 2], [1, T]])
                        rhs2 = bass.AP(tensor=Cc[:, :].tensor,
                                       offset=Cc[:, :].offset + jj * TP,
                                       ap=[[3 * BL, F], [BL, 2], [1, T]])
                        nc.tensor.matmul(Up[:, s * 2 * T:(s + 1) * 2 * T],
                                         Msb[:, (2 * j + 1) * F:(2 * j + 2) * F],
                                         rhs2, start=True, stop=False)
                        nc.tensor.matmul(Up[:, s * 2 * T:(s + 1) * 2 * T],
                                         Msb[:, (2 * j) * F:(2 * j + 1) * F],
                                         rhs1, start=False, stop=True)
                    # slot j = [ghost | t0..t50] at j*TP within each component
                    dst = bass.AP(tensor=Ue[c][:, :].tensor,
                                  offset=Ue[c][:, :].offset + bk * PBK * TP + 1,
                                  ap=[[2 * UEC, F], [TP, PBK], [UEC, 2], [1, T]])
                    nc.scalar.activation(
                        dst, Up[:, :].rearrange("p (s c2 t) -> p s c2 t", s=PBK, c2=2),
                        CPY)

            def u_ghost(c):
                """ghost_j <- data_j[t=50] for all (c2, j): one gpsimd copy."""
                gdst = bass.AP(tensor=Ue[c][:, :].tensor,
                               offset=Ue[c][:, :].offset,
                               ap=[[2 * UEC, F], [UEC, 2], [TP, CHJ]])
                gsrc = bass.AP(tensor=Ue[c][:, :].tensor,
                               offset=Ue[c][:, :].offset + T,
                               ap=[[2 * UEC, F], [UEC, 2], [TP, CHJ]])
                nc.gpsimd.tensor_copy(gdst, gsrc)

            def u_roll(c):
                """U = Ue[x+1] + Ue[x] flat per component (ghosts wrap t=0)."""
                dst = bass.AP(tensor=Usb[c][:, :].tensor,
                              offset=Usb[c][:, :].offset,
                              ap=[[2 * BL, F], [BL, 2], [1, BL]])
                s1 = bass.AP(tensor=Ue[c][:, :].tensor,
                             offset=Ue[c][:, :].offset + 1,
                             ap=[[2 * UEC, F], [UEC, 2], [1, BL]])
                s0 = bass.AP(tensor=Ue[c][:, :].tensor,
                             offset=Ue[c][:, :].offset,
                             ap=[[2 * UEC, F], [UEC, 2], [1, BL]])
                TT(dst, s1, s0, ADD)

            def v_stage(c):
                """V = U * R -> Vsb [Vr | Vi]; wide-packed TTs.

                sP = [a|b] = [Ur*Rr | Ui*Ri]; sQ = [cc|dd] = [Ur*Ri | Ui*Rr]
                Vr = a-b ; Vi = cc+dd.
                """
                Rc, Uc, Vc = Rsb[c], Usb[c], Vsb[c]
                # Vr first so the G stage's vr passes unblock before Vi exists
                TT(sP[c][:, :], rf(Uc), rf(Rc), MUL)
                TT(Vc[:, 0:BL], sP[c][:, 0:BL], sP[c][:, BL:2 * BL], SUB)
                TT(sQ[c][:, :], rf(Uc), rf(Rc, rev=True), MUL)
                TT(Vc[:, BL:2 * BL], sQ[c][:, 0:BL], sQ[c][:, BL:2 * BL], ADD)

            Dp = ps_d.tile([F, 2 * T], FP32, tag="Dp")

            def g_stage(c, start, stop):
                """D += sum_j G @ V_j : zero-stride dst accumulates j in PSUM."""
                Vc = Vsb[c]
                dstR = bass.AP(tensor=Dp[:, :].tensor, offset=Dp[:, :].offset,
                               ap=[[2 * T, F], [0, CHJ], [1, T]])
                dstI = bass.AP(tensor=Dp[:, :].tensor, offset=Dp[:, :].offset + T,
                               ap=[[2 * T, F], [0, CHJ], [1, T]])
                vr = bass.AP(tensor=Vc[:, :].tensor, offset=Vc[:, :].offset,
                             ap=[[2 * BL, F], [TP, CHJ], [1, T]])
                vi = bass.AP(tensor=Vc[:, :].tensor, offset=Vc[:, :].offset + BL,
                             ap=[[2 * BL, F], [TP, CHJ], [1, T]])

                if start:
                    dfull = bass.AP(tensor=Dp[:, :].tensor, offset=Dp[:, :].offset,
                                    ap=[[2 * T, F], [1, 2 * T]])
                    nc.tensor.matmul(dfull, Gc[:, 0:F], Zsb[:, :],
                                     start=True, stop=False,
                                     skip_group_check=True)
                nc.tensor.matmul(dstI, Gc[:, F:2 * F], vr,
                                 start=False, stop=False, skip_group_check=True)
                nc.tensor.matmul(dstR, Gc[:, 0:F], vr,
                                 start=False, stop=False, skip_group_check=True)
                nc.tensor.matmul(dstI, Gc[:, 0:F], vi,
                                 start=False, stop=False, skip_group_check=True)
                nc.tensor.matmul(dstR, Gc[:, 2 * F:3 * F], vi,
                                 start=False, stop=stop, skip_group_check=True)

            # ---------- pipelined issue order ----------
            r_stage(0, split_evict=True)
            r_stage(1)
            c_stage(0)
            u_mm(0)
            c_stage(1)
            u_ghost(0)
            u_roll(0)
            v_stage(0)
            u_mm(1)
            g_stage(0, start=True, stop=False)
            u_ghost(1)
            u_roll(1)
            v_stage(1)
            g_stage(1, start=False, stop=True)

            # ---------- tail: evict D (fp32) and DMA out ----------
            Dsb = wpool.tile([F, 2 * T], FP32, tag="Dsb")
            nc.scalar.activation(Dsb[:, :], Dp[:, :], CPY)
            nc.sync.dma_start(dv[:, :], Dsb[:, :])
    return nc


# ---------------- host side ----------------

def _host_consts():
    W, G = _dft_consts()
    fr_c = np.concatenate([W.real, W.imag], axis=1).astype(bfloat16)
    gr_c = np.concatenate([G.real, G.imag, -G.imag], axis=1).astype(bfloat16)
    idx = (np.arange(T)[:, None] * HOP + np.arange(F)[None, :]).reshape(-1)
    cov = np.zeros(L)
    np.add.at(cov, idx, 1.0)
    cov = np.where(cov > 0, cov, 1.0)
    return fr_c, gr_c, cov, idx


def _smat_for(n2_list):
    S = np.zeros((NJ, F, F), np.float32)
    g = np.arange(F)
    for j, n2 in enumerate(n2_list):
        S[j, (g - n2) % F, g] = 1.0
    return np.ascontiguousarray(
        S.transpose(1, 0, 2).reshape(F, NJ * F)).astype(float8_e4m3)


def _mst_for(n2_list, w2):
    Ms = np.zeros((NJ, 2, F, F), np.float32)
    g = np.arange(F)[:, None]
    f = np.arange(F)[None, :]
    n1 = ((f - g + 20) % F) - 20
    valid = (n1 >= -20) & (n1 <= 19)
    n1c = np.clip(n1 + 20, 0, 39)
    for j, n2 in enumerate(n2_list):
        col = w2[:, n2 + 20]
        Ms[j, 0] = np.where(valid, col.real[n1c], 0.0)
        Ms[j, 1] = np.where(valid, col.imag[n1c], 0.0)
    return np.ascontiguousarray(
        Ms.transpose(2, 0, 1, 3).reshape(F, NJ * 2 * F)).astype(bfloat16)


def _frame(sig):
    idx = np.arange(T)[None, :] * HOP + np.arange(F)[:, None]   # [j, t]
    return sig[idx].astype(np.float32)


def make_in_maps(x_real, x_imag, task_info, w_real, w_imag):
    fr_c, gr_c, cov, idx = _host_consts()
    b, _, m = x_real.shape
    P = np.power(10.0, task_info[:, 0] / 10.0) / m
    w2 = (np.asarray(w_real) + 1j * np.asarray(w_imag)).reshape(40, 40)
    smats = [_smat_for(nl) for nl in N2_LISTS]
    msts = [_mst_for(nl, w2) for nl in N2_LISTS]

    in_maps, shards = [], []
    for bb in range(b):
        for mm in range(m):
            fr_ = _frame(x_real[bb, :, mm])
            fi_ = _frame(x_imag[bb, :, mm])
            critv = np.concatenate(
                [np.concatenate([-fi_, fr_, fi_], axis=1).astype(bfloat16), fr_c],
                axis=1)
            for h in range(2):
                in_maps.append({
                    "crit": critv,
                    "gr_c": gr_c,
                    "smat": smats[h],
                    "mst": msts[h],
                })
                shards.append((bb, mm, h))
    return in_maps, shards, P, cov, idx


_NC_CACHE = {}


def kernel(x_real, x_imag, task_info, w_real, w_imag, b_real, b_imag):
    x_real = np.asarray(x_real)
    x_imag = np.asarray(x_imag)
    task_info = np.asarray(task_info)
    b, Lx, m = x_real.shape
    assert (b, Lx, m) == (2, L, 2)

    if "nc" not in _NC_CACHE:
        nc_ = build_program(debug=False)
        nc_.compile()
        _NC_CACHE["nc"] = nc_
    nc = _NC_CACHE["nc"]

    in_maps, shards, P, cov, idx = make_in_maps(
        x_real, x_imag, task_info, w_real, w_imag)
    from concourse.bass_utils import run_bass_kernel_spmd
    res = run_bass_kernel_spmd(nc, in_maps, list(range(8))).results

    x = (x_real + 1j * x_imag).astype(np.complex64)
    out = x.copy()
    bias = complex(np.asarray(b_real)[0], np.asarray(b_imag)[0])
    bias_sig = np.zeros(L, np.complex64)
    bias_sig[np.arange(T) * HOP] = bias
    bias_sig /= cov
    acc = np.zeros((b, m, L), np.complex128)
    for i, (bb, mm, h) in enumerate(shards):
        dvv = res[i]["dv"]          # [80, 102] = [s, (Dr(51) | Di(51))]
        d = dvv[:, 0:T] + 1j * dvv[:, T:2 * T]     # [s, t]
        np.add.at(acc[bb, mm], idx, d.T.reshape(-1))
    for bb in range(b):
        for mm in range(m):
            y = (acc[bb, mm] / cov) * P[bb]
            out[bb, :, mm] += y.astype(np.complex64)
            out[bb, :, mm] += (P[bb] * bias_sig).astype(np.complex64)
    return out[:, 20:L - 20, :]


# revision 11
# speedup vs baseline: 1.0688x; 1.0688x over previous
"""Trainium2 Bass kernel for nn_EqStftPBC (STFT perturbation-based compensation).

Per (batch b, mode m):
  X = STFT(x); C_n2 = X*conj(roll(X,n2)); U_n2 = circ(w[:,n2]) @ C (+ time-roll);
  V_n2 = U_n2 * roll(X,n2); delta_f = sum_n2 V_n2; D = IDFT(delta); host OLA.
8 cores = (b x m x n2-half); per-core data-only variation (S/M stacks).

v7 (from v5 ~31.5us):
- device outputs D [80, 2T] fp32; overlap-add/cov/P-scale moved to host
  (kills Y-stage mms + selector consts + D guard memsets; shorter tail).
- XtB broadcast ACT removed: C-stage reads X via 0-stride-over-j APs.
- input DMA posts moved off scalar (ACT_TABLE_LOAD no longer gates them):
  crit posted by the tensor engine itself at t~6.1us, smat by vector,
  mst/gr_c by gpsimd; sync carries ONLY the output DMA.
- component-major R/U/V layouts ([comp(520) | comp(520)]) enabling
  wide-packed C/V stages: 2 double-width MUL TTs + 2 combine TTs each
  (was 6 TTs) -- fewer DVE ops, same math.
- time-roll as before: ghost slots + one flat TT per chunk.
- G-stage j-sum in PSUM via zero-stride dst (tensor has slack vs DVE).
"""

import numpy as np
from ml_dtypes import bfloat16, float8_e4m3

import concourse.bass as bass
import concourse.bacc as bacc
import concourse.mybir as mybir
import concourse.tile as tile

F = 80
T = 51
TP = 52          # per-j slot stride (51 data + 1 pad)
HOP = 40
L = 2080
NJ = 20
NCH = 2
CHJ = NJ // NCH  # 10
PBK = 5          # j's per R/U psum bank
BL = CHJ * TP    # 520
UEC = CHJ * TP + 1   # 521: per-component Ue extent (slots + 1 tail junk)
FP32 = mybir.dt.float32
BF16 = mybir.dt.bfloat16
FP8 = mybir.dt.float8e4

N2_LISTS = [list(range(19, -1, -1)), list(range(-1, -21, -1))]


def _dft_consts():
    j = np.arange(F)
    W = np.exp(-2j * np.pi * np.outer(j, j) / F)
    G = np.exp(+2j * np.pi * np.outer(j, j) / F) / F
    return W, G


def build_program(debug=False):
    nc = bacc.Bacc("TRN2", target_bir_lowering=False, debug=debug)

    # crit = [xf frames (3T) | fr_c (2F)]: one DMA gates the STFT
    crit = nc.dram_tensor("crit", [F, 3 * T + 2 * F], BF16, kind="ExternalInput")
    # gr_c = [Gr | Gi | GiN]  (GiN = -Gi)
    gr_c = nc.dram_tensor("gr_c", [F, 3 * F], BF16, kind="ExternalInput")
    smat = nc.dram_tensor("smat", [F, NJ * F], FP8, kind="ExternalInput")
    mst = nc.dram_tensor("mst", [F, NJ * 2 * F], BF16, kind="ExternalInput")
    dv = nc.dram_tensor("dv", [F, 2 * T], FP32, kind="ExternalOutput")

    MUL = mybir.AluOpType.mult
    ADD = mybir.AluOpType.add
    SUB = mybir.AluOpType.subtract
    CPY = mybir.ActivationFunctionType.Copy

    with tile.TileContext(nc) as tc:
        with (
            tc.tile_pool(name="const", bufs=1) as cpool,
            tc.tile_pool(name="work", bufs=1) as wpool,
            tc.tile_pool(name="ps_x", bufs=1, space="PSUM") as ps_x,
            tc.tile_pool(name="ps_r", bufs=2, space="PSUM") as ps_r,
            tc.tile_pool(name="ps_u", bufs=2, space="PSUM") as ps_u,
            tc.tile_pool(name="ps_d", bufs=1, space="PSUM") as ps_d,
        ):
            # ---- input DMAs: only gpsimd/sync/scalar may post. crit alone on
            # the sync queue (fast first-post); everything else on the
            # high-bandwidth gpsimd queue, smat first (it gates the R stage);
            # scalar posts nothing so its ACTs are never queue-blocked.
            Crit = wpool.tile([F, 3 * T + 2 * F], BF16, tag="Crit")
            nc.sync.dma_start(Crit[:, :], crit[:, :])
            FCO = 3 * T   # Fc column offset within Crit
            # smat first on the fast gpsimd queue. The mst posts are held back
            # by a dummy gpsimd read of Ssb: if mst is in-queue while smat is
            # in flight, descriptor striping delays smat's last descriptor by
            # ~1.6us, gating the whole R stage.
            Ssb = cpool.tile([F, NJ * F], FP8, tag="Ssb")
            nc.gpsimd.dma_start(Ssb[:, 0:CHJ * F], smat[:, 0:CHJ * F])
            nc.gpsimd.dma_start(Ssb[:, CHJ * F:], smat[:, CHJ * F:])
            Sdum = wpool.tile([F, 2], FP8, tag="Sdum")
            nc.gpsimd.tensor_copy(
                Sdum[:, :],
                bass.AP(tensor=Ssb[:, :].tensor, offset=Ssb[:, :].offset + CHJ * F - 1,
                        ap=[[NJ * F, F], [CHJ * F, 2], [1, 1]]))
            Msb = cpool.tile([F, NJ * 2 * F], BF16, tag="Msb")
            nc.gpsimd.dma_start(Msb[:, 0:CHJ * 2 * F], mst[:, 0:CHJ * 2 * F])
            nc.gpsimd.dma_start(Msb[:, CHJ * 2 * F:], mst[:, CHJ * 2 * F:])
            Gc = cpool.tile([F, 3 * F], BF16, tag="Gc")
            nc.gpsimd.dma_start(Gc[:, :], gr_c[:, :])

            # zero rhs for the PSUM-accumulation opener matmul
            Zsb = wpool.tile([F, 2 * T], BF16, tag="Zsb")
            nc.gpsimd.memset(Zsb[:, :], 0.0)

            # ---- STFT (fp32 accum) -> X bf16 [Xr(52) | Xi(52)] ----
            Xp = ps_x.tile([F, 2 * T], FP32, tag="Xp")
            nc.tensor.matmul(Xp[:, :], Crit[:, FCO:FCO + F], Crit[:, T:3 * T],
                             start=True, stop=False)
            nc.tensor.matmul(Xp[:, :], Crit[:, FCO + F:FCO + 2 * F], Crit[:, 0:2 * T],
                             start=False, stop=True)
            Xsb = wpool.tile([F, 2 * TP], BF16, tag="Xsb")
            # pad columns of the X slots (read by the C-stage broadcast APs)
            nc.gpsimd.memset(bass.AP(tensor=Xsb[:, :].tensor,
                                     offset=Xsb[:, :].offset + T,
                                     ap=[[2 * TP, F], [TP, 2], [1, 1]]), 0.0)
            Xsv = Xsb[:, :].rearrange("p (c t) -> p c t", c=2)
            nc.scalar.activation(Xsv[:, :, 0:T],
                                 Xp[:, :].rearrange("p (c t) -> p c t", c=2), CPY)
            Xrhs = bass.AP(tensor=Xsb[:, :].tensor, offset=Xsb[:, :].offset,
                           ap=[[2 * TP, F], [TP, 2], [1, T]])

            # X broadcast APs for the C stage: (c2, j0-stride, t) and the
            # c2-reversed variant (Xi then Xr) for the P3/P4 products.
            def x_bcast(rev):
                if not rev:
                    return bass.AP(tensor=Xsb[:, :].tensor, offset=Xsb[:, :].offset,
                                   ap=[[2 * TP, F], [TP, 2], [0, CHJ], [1, TP]])
                return bass.AP(tensor=Xsb[:, :].tensor,
                               offset=Xsb[:, :].offset + TP,
                               ap=[[2 * TP, F], [-TP, 2], [0, CHJ], [1, TP]])

            # ---- per-chunk tiles (component-major: [r(520) | i(520)]) ----
            Rsb, Csb, Usb, Vsb, Ue = [], [], [], [], []
            for c in range(NCH):
                Rsb.append(wpool.tile([F, 2 * BL], BF16, tag=f"Rsb{c}", name=f"Rsb{c}"))
                Csb.append(wpool.tile([F, 3 * BL], BF16, tag=f"Csb{c}", name=f"Csb{c}"))
                Usb.append(wpool.tile([F, 2 * BL], BF16, tag=f"Usb{c}", name=f"Usb{c}"))
                Vsb.append(wpool.tile([F, 2 * BL], BF16, tag=f"Vsb{c}", name=f"Vsb{c}"))
                Ue.append(wpool.tile([F, 2 * UEC], BF16, tag=f"Ue{c}", name=f"Ue{c}"))
                # tail junk element per component (read by the roll TT pad col)
                nc.gpsimd.memset(bass.AP(tensor=Ue[c][:, :].tensor,
                                         offset=Ue[c][:, :].offset + UEC - 1,
                                         ap=[[2 * UEC, F], [UEC, 2], [1, 1]]), 0.0)
            sP = [wpool.tile([F, 2 * BL], BF16, tag=f"sP{c}", name=f"sP{c}")
                  for c in range(NCH)]
            sQ = [wpool.tile([F, 2 * BL], BF16, tag=f"sQ{c}", name=f"sQ{c}")
                  for c in range(NCH)]

            TT = nc.vector.tensor_tensor

            def r_stage(c, split_evict=False):
                """R_j = roll(X, n2_j): permutation matmuls, evict to SBUF.

                split_evict: bank B evicted by a vector tensor_copy so both
                bank evictions run concurrently (shorter pipeline fill).
                """
                Rc = Rsb[c]
                for bk in range(CHJ // PBK):
                    Rp = ps_r.tile([F, PBK * 2 * T], FP32, tag="Rp")
                    for s in range(PBK):
                        j = c * CHJ + bk * PBK + s
                        nc.tensor.matmul(Rp[:, s * 2 * T:(s + 1) * 2 * T],
                                         Ssb[:, j * F:(j + 1) * F],
                                         Xrhs, start=True, stop=True)
                    dst = bass.AP(tensor=Rc[:, :].tensor,
                                  offset=Rc[:, :].offset + bk * PBK * TP,
                                  ap=[[2 * BL, F], [TP, PBK], [BL, 2], [1, T]])
                    src = Rp[:, :].rearrange("p (s c2 t) -> p s c2 t", s=PBK, c2=2)
                    if split_evict and bk == 1:
                        nc.vector.tensor_copy(dst, src)
                    else:
                        nc.scalar.activation(dst, src, CPY)

            def rf(tile_, rev=False, off=0):
                """[comp0 | comp1] (or reversed) view of a component-major tile."""
                if not rev:
                    return bass.AP(tensor=tile_[:, :].tensor,
                                   offset=tile_[:, :].offset + off,
                                   ap=[[2 * BL, F], [BL, 2], [1, BL]])
                return bass.AP(tensor=tile_[:, :].tensor,
                               offset=tile_[:, :].offset + BL + off,
                               ap=[[2 * BL, F], [-BL, 2], [1, BL]])

            def c_stage(c):
                """C = X * conj(R) -> Csb [CiN | Cr | Ci]; wide-packed TTs.

                sP = [P1|P2] = [Xr*Rr | Xi*Ri]; sQ = [P3|P4] = [Xi*Rr | Xr*Ri]
                Cr = P1+P2 ; CiN = P4-P3 ; Ci = -CiN (scalar).
                """
                Rc, Cc = Rsb[c], Csb[c]
                TT(sQ[c][:, :], x_bcast(True), rf(Rc), MUL)
                TT(Cc[:, 0:BL], sQ[c][:, BL:2 * BL], sQ[c][:, 0:BL], SUB)
                nc.scalar.activation(Cc[:, 2 * BL:3 * BL], Cc[:, 0:BL], CPY,
                                     scale=-1.0)
                TT(sP[c][:, :], x_bcast(False), rf(Rc), MUL)
                TT(Cc[:, BL:2 * BL], sP[c][:, 0:BL], sP[c][:, BL:2 * BL], ADD)

            def u_mm(c):
                """Up_j = Mi@[CiN|Cr] + Mr@[Cr|Ci]; evict into ghost-slotted Ue."""
                Cc = Csb[c]
                for bk in range(CHJ // PBK):
                    Up = ps_u.tile([F, PBK * 2 * T], FP32, tag="Up")
                    for s in range(PBK):
                        jj = bk * PBK + s
                        j = c * CHJ + jj
                        rhs1 = bass.AP(tensor=Cc[:, :].tensor,
                                       offset=Cc[:, :].offset + BL + jj * TP,
                                       ap=[[3 * BL, F], [BL, 2], [1, T]])
                        rhs2 = bass.AP(tensor=Cc[:, :].tensor,
                                       offset=Cc[:, :].offset + jj * TP,
                                       ap=[[3 * BL, F], [BL, 2], [1, T]])
                        nc.tensor.matmul(Up[:, s * 2 * T:(s + 1) * 2 * T],
                                         Msb[:, (2 * j + 1) * F:(2 * j + 2) * F],
                                         rhs2, start=True, stop=False)
                        nc.tensor.matmul(Up[:, s * 2 * T:(s + 1) * 2 * T],
                                         Msb[:, (2 * j) * F:(2 * j + 1) * F],
                                         rhs1, start=False, stop=True)
                    # slot j = [ghost | t0..t50] at j*TP within each component
                    dst = bass.AP(tensor=Ue[c][:, :].tensor,
                                  offset=Ue[c][:, :].offset + bk * PBK * TP + 1,
                                  ap=[[2 * UEC, F], [TP, PBK], [UEC, 2], [1, T]])
                    nc.scalar.activation(
                        dst, Up[:, :].rearrange("p (s c2 t) -> p s c2 t", s=PBK, c2=2),
                        CPY)

            def u_ghost(c):
                """ghost_j <- data_j[t=50] for all (c2, j): one gpsimd copy."""
                gdst = bass.AP(tensor=Ue[c][:, :].tensor,
                               offset=Ue[c][:, :].offset,
                               ap=[[2 * UEC, F], [UEC, 2], [TP, CHJ]])
                gsrc = bass.AP(tensor=Ue[c][:, :].tensor,
                               offset=Ue[c][:, :].offset + T,
                               ap=[[2 * UEC, F], [UEC, 2], [TP, CHJ]])
                nc.gpsimd.tensor_copy(gdst, gsrc)

            def u_roll(c):
                """U = Ue[x+1] + Ue[x] flat per component (ghosts wrap t=0)."""
                dst = bass.AP(tensor=Usb[c][:, :].tensor,
                              offset=Usb[c][:, :].offset,
                              ap=[[2 * BL, F], [BL, 2], [1, BL]])
                s1 = bass.AP(tensor=Ue[c][:, :].tensor,
                             offset=Ue[c][:, :].offset + 1,
                             ap=[[2 * UEC, F], [UEC, 2], [1, BL]])
                s0 = bass.AP(tensor=Ue[c][:, :].tensor,
                             offset=Ue[c][:, :].offset,
                             ap=[[2 * UEC, F], [UEC, 2], [1, BL]])
                TT(dst, s1, s0, ADD)

            def v_stage(c):
                """V = U * R -> Vsb [Vr | Vi]; wide-packed TTs.

                sP = [a|b] = [Ur*Rr | Ui*Ri]; sQ = [cc|dd] = [Ur*Ri | Ui*Rr]
                Vr = a-b ; Vi = cc+dd.
                """
                Rc, Uc, Vc = Rsb[c], Usb[c], Vsb[c]
                # Vr first so the G stage's vr passes unblock before Vi exists
                TT(sP[c][:, :], rf(Uc), rf(Rc), MUL)
                TT(Vc[:, 0:BL], sP[c][:, 0:BL], sP[c][:, BL:2 * BL], SUB)
                TT(sQ[c][:, :], rf(Uc), rf(Rc, rev=True), MUL)
                TT(Vc[:, BL:2 * BL], sQ[c][:, 0:BL], sQ[c][:, BL:2 * BL], ADD)

            Dp = ps_d.tile([F, 2 * T], FP32, tag="Dp")

            def g_stage(c, start, stop):
                """D += sum_j G @ V_j : zero-stride dst accumulates j in PSUM."""
                Vc = Vsb[c]
                dstR = bass.AP(tensor=Dp[:, :].tensor, offset=Dp[:, :].offset,
                               ap=[[2 * T, F], [0, CHJ], [1, T]])
                dstI = bass.AP(tensor=Dp[:, :].tensor, offset=Dp[:, :].offset + T,
                               ap=[[2 * T, F], [0, CHJ], [1, T]])
                vr = bass.AP(tensor=Vc[:, :].tensor, offset=Vc[:, :].offset,
                             ap=[[2 * BL, F], [TP, CHJ], [1, T]])
                vi = bass.AP(tensor=Vc[:, :].tensor, offset=Vc[:, :].offset + BL,
                             ap=[[2 * BL, F], [TP, CHJ], [1, T]])

                if start:
                    dfull = bass.AP(tensor=Dp[:, :].tensor, offset=Dp[:, :].offset,
                                    ap=[[2 * T, F], [1, 2 * T]])
                    nc.tensor.matmul(dfull, Gc[:, 0:F], Zsb[:, :],
                                     start=True, stop=False,
                                     skip_group_check=True)
                nc.tensor.matmul(dstI, Gc[:, F:2 * F], vr,
                                 start=False, stop=False, skip_group_check=True)
                nc.tensor.matmul(dstR, Gc[:, 0:F], vr,
                                 start=False, stop=False, skip_group_check=True)
                nc.tensor.matmul(dstI, Gc[:, 0:F], vi,
                                 start=False, stop=False, skip_group_check=True)
                nc.tensor.matmul(dstR, Gc[:, 2 * F:3 * F], vi,
                                 start=False, stop=stop, skip_group_check=True)

            # ---------- pipelined issue order ----------
            r_stage(0, split_evict=True)
            r_stage(1)
            c_stage(0)
            u_mm(0)
            c_stage(1)
            u_ghost(0)
            u_roll(0)
            v_stage(0)
            u_mm(1)
            g_stage(0, start=True, stop=False)
            u_ghost(1)
            u_roll(1)
            v_stage(1)
            g_stage(1, start=False, stop=True)

            # ---------- tail: evict D (fp32) and DMA out ----------
            Dsb = wpool.tile([F, 2 * T], FP32, tag="Dsb")
            nc.scalar.activation(Dsb[:, :], Dp[:, :], CPY)
            nc.sync.dma_start(dv[:, :], Dsb[:, :])
    return nc


# ---------------- host side ----------------

def _host_consts():
    W, G = _dft_consts()
    fr_c = np.concatenate([W.real, W.imag], axis=1).astype(bfloat16)
    gr_c = np.concatenate([G.real, G.imag, -G.imag], axis=1).astype(bfloat16)
    idx = (np.arange(T)[:, None] * HOP + np.arange(F)[None, :]).reshape(-1)
    cov = np.zeros(L)
    np.add.at(cov, idx, 1.0)
    cov = np.where(cov > 0, cov, 1.0)
    return fr_c, gr_c, cov, idx


def _smat_for(n2_list):
    S = np.zeros((NJ, F, F), np.float32)
    g = np.arange(F)
    for j, n2 in enumerate(n2_list):
        S[j, (g - n2) % F, g] = 1.0
    return np.ascontiguousarray(
        S.transpose(1, 0, 2).reshape(F, NJ * F)).astype(float8_e4m3)


def _mst_for(n2_list, w2):
    Ms = np.zeros((NJ, 2, F, F), np.float32)
    g = np.arange(F)[:, None]
    f = np.arange(F)[None, :]
    n1 = ((f - g + 20) % F) - 20
    valid = (n1 >= -20) & (n1 <= 19)
    n1c = np.clip(n1 + 20, 0, 39)
    for j, n2 in enumerate(n2_list):
        col = w2[:, n2 + 20]
        Ms[j, 0] = np.where(valid, col.real[n1c], 0.0)
        Ms[j, 1] = np.where(valid, col.imag[n1c], 0.0)
    return np.ascontiguousarray(
        Ms.transpose(2, 0, 1, 3).reshape(F, NJ * 2 * F)).astype(bfloat16)


def _frame(sig):
    idx = np.arange(T)[None, :] * HOP + np.arange(F)[:, None]   # [j, t]
    return sig[idx].astype(np.float32)


def make_in_maps(x_real, x_imag, task_info, w_real, w_imag):
    fr_c, gr_c, cov, idx = _host_consts()
    b, _, m = x_real.shape
    P = np.power(10.0, task_info[:, 0] / 10.0) / m
    w2 = (np.asarray(w_real) + 1j * np.asarray(w_imag)).reshape(40, 40)
    smats = [_smat_for(nl) for nl in N2_LISTS]
    msts = [_mst_for(nl, w2) for nl in N2_LISTS]

    in_maps, shards = [], []
    for bb in range(b):
        for mm in range(m):
            fr_ = _frame(x_real[bb, :, mm])
            fi_ = _frame(x_imag[bb, :, mm])
            critv = np.concatenate(
                [np.concatenate([-fi_, fr_, fi_], axis=1).astype(bfloat16), fr_c],
                axis=1)
            for h in range(2):
                in_maps.append({
                    "crit": critv,
                    "gr_c": gr_c,
                    "smat": smats[h],
                    "mst": msts[h],
                })
                shards.append((bb, mm, h))
    return in_maps, shards, P, cov, idx


_NC_CACHE = {}


def kernel(x_real, x_imag, task_info, w_real, w_imag, b_real, b_imag):
    x_real = np.asarray(x_real)
    x_imag = np.asarray(x_imag)
    task_info = np.asarray(task_info)
    b, Lx, m = x_real.shape
    assert (b, Lx, m) == (2, L, 2)

    if "nc" not in _NC_CACHE:
        nc_ = build_program(debug=False)
        nc_.compile()
        _NC_CACHE["nc"] = nc_
    nc = _NC_CACHE["nc"]

    in_maps, shards, P, cov, idx = make_in_maps(
        x_real, x_imag, task_info, w_real, w_imag)
    from concourse.bass_utils import run_bass_kernel_spmd
    res = run_bass_kernel_spmd(nc, in_maps, list(range(8))).results

    x = (x_real + 1j * x_imag).astype(np.complex64)
    out = x.copy()
    bias = complex(np.asarray(b_real)[0], np.asarray(b_imag)[0])
    bias_sig = np.zeros(L, np.complex64)
    bias_sig[np.arange(T) * HOP] = bias
    bias_sig /= cov
    acc = np.zeros((b, m, L), np.complex128)
    for i, (bb, mm, h) in enumerate(shards):
        dvv = res[i]["dv"]          # [80, 102] = [s, (Dr(51) | Di(51))]
        d = dvv[:, 0:T] + 1j * dvv[:, T:2 * T]     # [s, t]
        np.add.at(acc[bb, mm], idx, d.T.reshape(-1))
    for bb in range(b):
        for mm in range(m):
            y = (acc[bb, mm] / cov) * P[bb]
            out[bb, :, mm] += y.astype(np.complex64)
            out[bb, :, mm] += (P[bb] * bias_sig).astype(np.complex64)
    return out[:, 20:L - 20, :]
